# revision 38
# baseline (speedup 1.0000x reference)
"""3D Haar DWT (clean-mode subband stack) on 8 Trainium2 NeuronCores.

Problem (hardcoded): inputs (4, 128, 128, 128, 4) f32, A (128, 128) f32 Haar
analysis operator. Output (4, 64, 64, 64, 32) f32 = 8 subbands stacked on the
channel axis (LLL, LLH, LHL, LHH, HLL, HLH, HHL, HHH) x 4 channels.

Sharding: pure data parallel over (batch, d1-half): core k handles
b = k // 2, d1 range [64*(k%2), 64*(k%2)+64). The Haar transform is a 2-tap
non-overlapping filter, so splitting d1 on an even boundary requires no
communication.

The kernel is memory-bound (HBM ~358 GB/s per core), so the datapath runs in
bf16: the host uploads the input slab as bf16 (8 MiB/core instead of 16) and
the output is stored as bf16 (8 MiB instead of 16), halving HBM traffic vs
f32. absmax-relative error lands ~4e-3, inside the 2e-2 gate.

Key structure: the partition axis carries (o1_sub 16, d1-member, d3-parity,
d2-member) = 128, so a SINGLE PE pass applies ALL THREE Haar butterflies at
once — the stationary matrix is the triple Kronecker of the 2-tap stages
(entries +-0.25, bf16-exact, 8 nonzeros per output, loaded once). The PSUM
partition axis comes out subband-major: (s1, s3, s2, o1_sub). The ONLY
elementwise work left is the mandatory one-input PSUM evacuation (the ISA
allows at most one PSUM operand per elementwise op and DMA cannot read
PSUM), split 50/50 between ACT and DVE (~19 us each) — every engine then
sits far below the ~42 us DMA stream, so the pipeline is DMA-ring-paced
end to end.

Per-core pipeline (host layout [(o1s,m1,m3,m2) = 128, o3 64, o1blk 2,
o2 64, c 4]):
  1. DMA in 2 MiB loads (16 KiB descriptor runs — measured 399 GB/s vs 326
     at 1 MiB) on the SP HWDGE ring; all loads enqueued before any store so
     stores can never head-of-line-block a load.
  2. PE: 8 x 512-col matmuls per 8-o3 block with the Kronecker stationary
     (one PSUM bank each; the f32-PSUM moving-operand ISA cap is 512 cols).
  3. PSUM evacuation (one-input copy, f32 -> bf16) into the store staging
     tile, one op per half-block, alternating ACT / DVE.
  4. One 1 MiB store per block (8 KiB runs) on the SP ring; the output
     partition axis is already subband-major, so the host just casts and
     transposes.

Scale bookkeeping: reference applies s = 1/sqrt(2) per axis (s^3 total). The
host pre-scales by sqrt(2) and the PE applies 0.25: sqrt(2)/4 = s^3 — exact.
All three butterflies accumulate in f32 PSUM, so the only roundings are the
input cast and the evacuation cast.
"""

import sys

import numpy as np

if "/opt/trn_rl_repo" not in sys.path:
    sys.path.insert(0, "/opt/trn_rl_repo")

B, N, C = 4, 128, 4
N_CORES = 8
SLAB = 64          # d1 extent per core
LO3 = 16           # o3 values per load (2 MiB transfers, 16 KiB runs)
O3C = 8            # o3 values per compute block / store (1 MiB stores)
NLOAD = 64 // LO3
NBLK = 64 // O3C

_BASS_CACHE = {}


def _haar_matrix():
    s = np.float32(1.0 / np.sqrt(2.0))
    A = np.zeros((N, N), dtype=np.float32)
    for i in range(N // 2):
        A[i, 2 * i] = s
        A[i, 2 * i + 1] = s
        A[64 + i, 2 * i] = -s
        A[64 + i, 2 * i + 1] = s
    return A


def _kron_weights():
    """lhsT [p_in, p_out] for the combined (d1, d3, d2) butterfly.

    p_in  = 8 * o1s + 4 * m1 + 2 * m3 + m2   (input partition order)
    p_out = 64 * s1 + 32 * s3 + 16 * s2 + o1s (output partition order)
    weight = 0.25 * g(s1, m1) * g(s3, m3) * g(s2, m2),
    g(0, m) = +1, g(1, 0) = -1, g(1, 1) = +1 (Haar lo = a+b, hi = b-a).
    """
    g = np.array([[1.0, 1.0], [-1.0, 1.0]], dtype=np.float32)
    lhsT = np.zeros((N, N), dtype=np.float32)
    for o1s in range(16):
        for m1 in range(2):
            for m3 in range(2):
                for m2 in range(2):
                    p_in = 8 * o1s + 4 * m1 + 2 * m3 + m2
                    for s1 in range(2):
                        for s3 in range(2):
                            for s2 in range(2):
                                p_out = 64 * s1 + 32 * s3 + 16 * s2 + o1s
                                lhsT[p_in, p_out] = (
                                    0.25 * g[s1, m1] * g[s3, m3] * g[s2, m2]
                                )
    return lhsT


def _reference_numpy(inputs, A):
    # Fallback only: exact reference math on host (used if A is not Haar).
    x = np.einsum("ij,bpjqc->bpiqc", A, inputs)
    x = np.einsum("ij,bjpqc->bipqc", A, x)
    x = np.einsum("ij,bpqjc->bpqic", A, x)
    m = x.shape[1] // 2
    subs = [
        x[:, :m, :m, :m, :], x[:, :m, :m, m:, :],
        x[:, :m, m:, :m, :], x[:, :m, m:, m:, :],
        x[:, m:, :m, :m, :], x[:, m:, :m, m:, :],
        x[:, m:, m:, :m, :], x[:, m:, m:, m:, :],
    ]
    return np.concatenate(subs, axis=-1).astype(np.float32)


def _build_bass():
    import concourse.bacc as bacc
    import concourse.mybir as mybir
    import concourse.tile as tile

    f32 = mybir.dt.float32
    bf16 = mybir.dt.bfloat16

    # Bacc (not raw Bass): its compile() pipeline splits multi-sem waits into
    # EventSemaphore instructions — TRN2 instructions have one wait slot.
    nc = bacc.Bacc("TRN2", target_bir_lowering=False, debug=False)
    # x host layout: [(o1s,m1,m3,m2), o3, o1blk, o2, c]; each load descriptor
    # covers a 16 KiB contiguous run per partition.
    x = nc.dram_tensor("x", [N, 64, 2, 64, C], bf16, kind="ExternalInput")
    wk = nc.dram_tensor("wk", [N, N], bf16, kind="ExternalInput")
    # y: [(s1, s3, s2, o1s) = 128, o3, o1blk, o2, c]; per-partition
    # contiguous run for one block's o3 range = 8 KiB.
    y = nc.dram_tensor("y", [N, 64, 2, 64, C], bf16, kind="ExternalOutput")

    with tile.TileContext(nc) as tc:
        with (
            tc.tile_pool(name="const", bufs=1) as cpool,
            tc.tile_pool(name="io", bufs=4) as tpool,
            tc.tile_pool(name="wmid", bufs=4) as wpool,
            tc.tile_pool(name="psum", bufs=4, space="PSUM") as ppool,
        ):
            wk_sb = cpool.tile([N, N], bf16)

            # 1. all loads enqueued up-front on the SP ring: none depends on
            # compute (pool depth covers every load), so the load stream
            # runs back-to-back from the end of the preamble. The first load
            # is 1 MiB so the PE starts ~2.5 us earlier — in fast-DMA device
            # states the drain is PE-paced, so the whole compute stream
            # shifting earlier closes the trailing DMA gaps.
            spans = [(0, 8), (8, 16), (16, 32), (32, 48), (48, 64)]
            Ts = []
            for li, (a, b) in enumerate(spans):
                T = tpool.tile([N, b - a, 2 * 64 * C], bf16, tag=f"T{b - a}")
                nc.sync.dma_start(
                    out=T[:],
                    in_=x[:, a:b].rearrange("p a k q c -> p a (k q c)"),
                )
                Ts.append((a, b, T))
                if li == 0:
                    nc.sync.dma_start(out=wk_sb[:], in_=wk[:, :])

            evac_t = 0
            for ci in range(NBLK):
                a, b, T = next(s for s in Ts if s[0] <= ci * O3C < s[1])
                off = ci * O3C - a
                # W: store staging, (p_out, o3, o1blk*o2*c) bf16.
                W = wpool.tile([N, O3C, 2 * 64 * C], bf16, tag="W")
                for hq in range(4):
                    # 2. all three butterflies as one matmul per o3 value:
                    # 512 cols -> one PSUM bank. PSUM tiles are 2 banks x
                    # 4 bufs: with 4 tiles in flight the evacuations
                    # (alternating engines) always keep up with the PE, so
                    # the matmul cadence never degrades to the evac rate.
                    ps = ppool.tile([N, 2, 512], f32, tag="ps")
                    for j in range(2):
                        nc.tensor.matmul(
                            ps[:, j],
                            lhsT=wk_sb[:],
                            rhs=T[:, off + 2 * hq + j],
                            start=True, stop=True,
                        )
                    # 3. one-input PSUM evacuation (f32 -> bf16) straight
                    # into the store staging tile, alternating ACT / DVE
                    # (both ~1.1 ns/elem per partition; 50/50 keeps either
                    # engine at ~19 us, far below the DMA stream).
                    dst = W[:, 2 * hq:2 * hq + 2]
                    if evac_t % 2 == 1:
                        nc.vector.tensor_copy(out=dst, in_=ps[:])
                    else:
                        nc.scalar.copy(out=dst, in_=ps[:])
                    evac_t += 1

                # 4. one 1 MiB store per block (8 KiB runs/partition) on the
                # SP ring, enqueued behind all loads. Serialized read-phase
                # then write-phase on one FIFO ring measured faster than
                # overlapping R/W on two rings (53.0 vs 57.2 us at equal
                # device rates — mixed traffic gains nothing and the
                # ACT-ring store dispatch trails compute).
                nc.sync.dma_start(
                    out=y[:, ci * O3C:(ci + 1) * O3C].rearrange(
                        "p a k q c -> p a (k q c)"
                    ),
                    in_=W[:],
                )
    nc.compile()
    return nc


def _prepare(x, A):
    """Host-side prep shared with test.py: build (nc, in_maps)."""
    import ml_dtypes

    if "nc" not in _BASS_CACHE:
        _BASS_CACHE["nc"] = _build_bass()
    nc = _BASS_CACHE["nc"]

    wk = np.ascontiguousarray(_kron_weights().astype(ml_dtypes.bfloat16))
    # pre-scale by sqrt(2): the PE applies 0.25 across the three butterflies,
    # so each path nets sqrt(2)/4 = (1/sqrt(2))^3.
    xb = (x * np.float32(np.sqrt(2.0))).astype(ml_dtypes.bfloat16)
    in_maps = []
    for k in range(N_CORES):
        b, h = divmod(k, 2)
        # slab [d1l 64, d2 128, d3 128, c] ->
        # [(o1s, m1, m3, m2) 128, o3 64, o1blk 2, o2 64, c]
        s = xb[b, h * SLAB:(h + 1) * SLAB]
        s = s.reshape(2, 16, 2, 64, 2, 64, 2, C)
        # axes: (o1blk 0, o1s 1, m1 2, o2 3, m2 4, o3 5, m3 6, c 7)
        s = s.transpose(1, 2, 6, 4, 5, 0, 3, 7)
        in_maps.append(
            {
                "x": np.ascontiguousarray(s.reshape(N, 64, 2, 64, C)),
                "wk": wk,
            }
        )
    return nc, in_maps


def _assemble(results):
    """Gather per-core bf16 y tensors into the full f32 output."""
    out = np.empty((B, 64, 64, 64, 8 * C), np.float32)
    for k in range(N_CORES):
        b, h = divmod(k, 2)
        # y: [(s1, s3, s2, o1s), o3, o1blk, o2, c]
        arr = results[k]["y"].astype(np.float32).reshape(
            2, 2, 2, 16, 64, 2, 64, C
        )
        # (s1 0, s3 1, s2 2, o1s 3, o3 4, o1blk 5, o2 6, c 7)
        #   -> (o1blk, o1s, o2, o3, s1, s2, s3, c)
        out[b, 32 * h:32 * h + 32] = (
            arr.transpose(5, 3, 6, 4, 0, 2, 1, 7).reshape(32, 64, 64, 8 * C)
        )
    return out


def kernel(**inputs):
    x = np.ascontiguousarray(np.asarray(inputs["inputs"], dtype=np.float32))
    A = np.asarray(inputs["A"], dtype=np.float32)
    assert x.shape == (B, N, N, N, C), x.shape

    if not np.allclose(A, _haar_matrix(), atol=1e-5):
        # Kernel hardcodes the 2-tap Haar structure; fall back for generic A.
        return _reference_numpy(x, A)

    from concourse.bass_utils import run_bass_kernel_spmd

    nc, in_maps = _prepare(x, A)
    res = run_bass_kernel_spmd(nc, in_maps, core_ids=list(range(N_CORES)))
    return _assemble(res.results)


# revision 39
# speedup vs baseline: 1.0060x; 1.0060x over previous
"""3D Haar DWT (clean-mode subband stack) on 8 Trainium2 NeuronCores.

Problem (hardcoded): inputs (4, 128, 128, 128, 4) f32, A (128, 128) f32 Haar
analysis operator. Output (4, 64, 64, 64, 32) f32 = 8 subbands stacked on the
channel axis (LLL, LLH, LHL, LHH, HLL, HLH, HHL, HHH) x 4 channels.

Sharding: pure data parallel over (batch, d1-half): core k handles
b = k // 2, d1 range [64*(k%2), 64*(k%2)+64). The Haar transform is a 2-tap
non-overlapping filter, so splitting d1 on an even boundary requires no
communication.

The kernel is memory-bound (HBM ~358 GB/s per core), so the datapath runs in
bf16: the host uploads the input slab as bf16 (8 MiB/core instead of 16) and
the output is stored as bf16 (8 MiB instead of 16), halving HBM traffic vs
f32. absmax-relative error lands ~4e-3, inside the 2e-2 gate.

Key structure: the partition axis carries (o1_sub 16, d1-member, d3-parity,
d2-member) = 128, so a SINGLE PE pass applies ALL THREE Haar butterflies at
once — the stationary matrix is the triple Kronecker of the 2-tap stages
(entries +-0.25, bf16-exact, 8 nonzeros per output, loaded once). The PSUM
partition axis comes out subband-major: (s1, s3, s2, o1_sub). The ONLY
elementwise work left is the mandatory one-input PSUM evacuation (the ISA
allows at most one PSUM operand per elementwise op and DMA cannot read
PSUM), split 50/50 between ACT and DVE (~19 us each) — every engine then
sits far below the ~42 us DMA stream, so the pipeline is DMA-ring-paced
end to end.

Per-core pipeline (host layout [(o1s,m1,m3,m2) = 128, o3 64, o1blk 2,
o2 64, c 4]):
  1. DMA in 2 MiB loads (16 KiB descriptor runs — measured 399 GB/s vs 326
     at 1 MiB) on the SP HWDGE ring; all loads enqueued before any store so
     stores can never head-of-line-block a load.
  2. PE: 8 x 512-col matmuls per 8-o3 block with the Kronecker stationary
     (one PSUM bank each; the f32-PSUM moving-operand ISA cap is 512 cols).
  3. PSUM evacuation (one-input copy, f32 -> bf16) into the store staging
     tile, one op per half-block, alternating ACT / DVE.
  4. One 1 MiB store per block (8 KiB runs) on the SP ring; the output
     partition axis is already subband-major, so the host just casts and
     transposes.

Scale bookkeeping: reference applies s = 1/sqrt(2) per axis (s^3 total). The
host pre-scales by sqrt(2) and the PE applies 0.25: sqrt(2)/4 = s^3 — exact.
All three butterflies accumulate in f32 PSUM, so the only roundings are the
input cast and the evacuation cast.
"""

import sys

import numpy as np

if "/opt/trn_rl_repo" not in sys.path:
    sys.path.insert(0, "/opt/trn_rl_repo")

B, N, C = 4, 128, 4
N_CORES = 8
SLAB = 64          # d1 extent per core
LO3 = 16           # o3 values per load (2 MiB transfers, 16 KiB runs)
O3C = 8            # o3 values per compute block / store (1 MiB stores)
NLOAD = 64 // LO3
NBLK = 64 // O3C

_BASS_CACHE = {}


def _haar_matrix():
    s = np.float32(1.0 / np.sqrt(2.0))
    A = np.zeros((N, N), dtype=np.float32)
    for i in range(N // 2):
        A[i, 2 * i] = s
        A[i, 2 * i + 1] = s
        A[64 + i, 2 * i] = -s
        A[64 + i, 2 * i + 1] = s
    return A


def _kron_weights():
    """lhsT [p_in, p_out] for the combined (d1, d3, d2) butterfly.

    p_in  = 8 * o1s + 4 * m1 + 2 * m3 + m2   (input partition order)
    p_out = 64 * s1 + 32 * s3 + 16 * s2 + o1s (output partition order)
    weight = 0.25 * g(s1, m1) * g(s3, m3) * g(s2, m2),
    g(0, m) = +1, g(1, 0) = -1, g(1, 1) = +1 (Haar lo = a+b, hi = b-a).
    """
    g = np.array([[1.0, 1.0], [-1.0, 1.0]], dtype=np.float32)
    lhsT = np.zeros((N, N), dtype=np.float32)
    for o1s in range(16):
        for m1 in range(2):
            for m3 in range(2):
                for m2 in range(2):
                    p_in = 8 * o1s + 4 * m1 + 2 * m3 + m2
                    for s1 in range(2):
                        for s3 in range(2):
                            for s2 in range(2):
                                p_out = 64 * s1 + 32 * s3 + 16 * s2 + o1s
                                lhsT[p_in, p_out] = (
                                    0.25 * g[s1, m1] * g[s3, m3] * g[s2, m2]
                                )
    return lhsT


def _reference_numpy(inputs, A):
    # Fallback only: exact reference math on host (used if A is not Haar).
    x = np.einsum("ij,bpjqc->bpiqc", A, inputs)
    x = np.einsum("ij,bjpqc->bipqc", A, x)
    x = np.einsum("ij,bpqjc->bpqic", A, x)
    m = x.shape[1] // 2
    subs = [
        x[:, :m, :m, :m, :], x[:, :m, :m, m:, :],
        x[:, :m, m:, :m, :], x[:, :m, m:, m:, :],
        x[:, m:, :m, :m, :], x[:, m:, :m, m:, :],
        x[:, m:, m:, :m, :], x[:, m:, m:, m:, :],
    ]
    return np.concatenate(subs, axis=-1).astype(np.float32)


def _build_bass():
    import concourse.bacc as bacc
    import concourse.mybir as mybir
    import concourse.tile as tile

    f32 = mybir.dt.float32
    bf16 = mybir.dt.bfloat16

    # Bacc (not raw Bass): its compile() pipeline splits multi-sem waits into
    # EventSemaphore instructions — TRN2 instructions have one wait slot.
    nc = bacc.Bacc("TRN2", target_bir_lowering=False, debug=False)
    # x host layout: [(o1s,m1,m3,m2), o3, o1blk, o2, c]; each load descriptor
    # covers a 16 KiB contiguous run per partition.
    x = nc.dram_tensor("x", [N, 64, 2, 64, C], bf16, kind="ExternalInput")
    wk = nc.dram_tensor("wk", [N, N], bf16, kind="ExternalInput")
    # y: [(s1, s3, s2, o1s) = 128, o3, o1blk, o2, c]; per-partition
    # contiguous run for one block's o3 range = 8 KiB.
    y = nc.dram_tensor("y", [N, 64, 2, 64, C], bf16, kind="ExternalOutput")

    with tile.TileContext(nc) as tc:
        with (
            tc.tile_pool(name="const", bufs=1) as cpool,
            tc.tile_pool(name="io", bufs=4) as tpool,
            tc.tile_pool(name="wmid", bufs=4) as wpool,
            tc.tile_pool(name="psum", bufs=4, space="PSUM") as ppool,
        ):
            wk_sb = cpool.tile([N, N], bf16)

            # 1. all loads enqueued up-front on the SP ring: none depends on
            # compute (pool depth covers every load), so the load stream
            # runs back-to-back from the end of the preamble. The first load
            # is 1 MiB so the PE starts ~2.5 us earlier — in fast-DMA device
            # states the drain is PE-paced, so the whole compute stream
            # shifting earlier closes the trailing DMA gaps.
            # loads alternate between the two HWDGE rings (SP and ACT) so two
            # read transfers are always in flight — deeper read queueing to
            # hide HBM read latency (the read phase lags the write phase,
            # 385 vs 415 GB/s, on a single sequential ring).
            spans = [(0, 8), (8, 16), (16, 32), (32, 48), (48, 64)]
            Ts = []
            for li, (a, b) in enumerate(spans):
                T = tpool.tile([N, b - a, 2 * 64 * C], bf16, tag=f"T{b - a}")
                eng = nc.sync if li % 2 == 0 else nc.scalar
                eng.dma_start(
                    out=T[:],
                    in_=x[:, a:b].rearrange("p a k q c -> p a (k q c)"),
                )
                Ts.append((a, b, T))
                if li == 0:
                    nc.sync.dma_start(out=wk_sb[:], in_=wk[:, :])

            evac_t = 0
            for ci in range(NBLK):
                a, b, T = next(s for s in Ts if s[0] <= ci * O3C < s[1])
                off = ci * O3C - a
                # W: store staging, (p_out, o3, o1blk*o2*c) bf16.
                W = wpool.tile([N, O3C, 2 * 64 * C], bf16, tag="W")
                for hq in range(4):
                    # 2. all three butterflies as one matmul per o3 value:
                    # 512 cols -> one PSUM bank. PSUM tiles are 2 banks x
                    # 4 bufs: with 4 tiles in flight the evacuations
                    # (alternating engines) always keep up with the PE, so
                    # the matmul cadence never degrades to the evac rate.
                    ps = ppool.tile([N, 2, 512], f32, tag="ps")
                    for j in range(2):
                        nc.tensor.matmul(
                            ps[:, j],
                            lhsT=wk_sb[:],
                            rhs=T[:, off + 2 * hq + j],
                            start=True, stop=True,
                        )
                    # 3. one-input PSUM evacuation (f32 -> bf16) straight
                    # into the store staging tile, alternating ACT / DVE
                    # (both ~1.1 ns/elem per partition; 50/50 keeps either
                    # engine at ~19 us, far below the DMA stream).
                    dst = W[:, 2 * hq:2 * hq + 2]
                    if evac_t % 2 == 1:
                        nc.vector.tensor_copy(out=dst, in_=ps[:])
                    else:
                        nc.scalar.copy(out=dst, in_=ps[:])
                    evac_t += 1

                # 4. one 1 MiB store per block (8 KiB runs/partition) on the
                # SP ring, enqueued behind all loads. Serialized read-phase
                # then write-phase on one FIFO ring measured faster than
                # overlapping R/W on two rings (53.0 vs 57.2 us at equal
                # device rates — mixed traffic gains nothing and the
                # ACT-ring store dispatch trails compute).
                nc.sync.dma_start(
                    out=y[:, ci * O3C:(ci + 1) * O3C].rearrange(
                        "p a k q c -> p a (k q c)"
                    ),
                    in_=W[:],
                )
    nc.compile()
    return nc


def _prepare(x, A):
    """Host-side prep shared with test.py: build (nc, in_maps)."""
    import ml_dtypes

    if "nc" not in _BASS_CACHE:
        _BASS_CACHE["nc"] = _build_bass()
    nc = _BASS_CACHE["nc"]

    wk = np.ascontiguousarray(_kron_weights().astype(ml_dtypes.bfloat16))
    # pre-scale by sqrt(2): the PE applies 0.25 across the three butterflies,
    # so each path nets sqrt(2)/4 = (1/sqrt(2))^3.
    xb = (x * np.float32(np.sqrt(2.0))).astype(ml_dtypes.bfloat16)
    in_maps = []
    for k in range(N_CORES):
        b, h = divmod(k, 2)
        # slab [d1l 64, d2 128, d3 128, c] ->
        # [(o1s, m1, m3, m2) 128, o3 64, o1blk 2, o2 64, c]
        s = xb[b, h * SLAB:(h + 1) * SLAB]
        s = s.reshape(2, 16, 2, 64, 2, 64, 2, C)
        # axes: (o1blk 0, o1s 1, m1 2, o2 3, m2 4, o3 5, m3 6, c 7)
        s = s.transpose(1, 2, 6, 4, 5, 0, 3, 7)
        in_maps.append(
            {
                "x": np.ascontiguousarray(s.reshape(N, 64, 2, 64, C)),
                "wk": wk,
            }
        )
    return nc, in_maps


def _assemble(results):
    """Gather per-core bf16 y tensors into the full f32 output."""
    out = np.empty((B, 64, 64, 64, 8 * C), np.float32)
    for k in range(N_CORES):
        b, h = divmod(k, 2)
        # y: [(s1, s3, s2, o1s), o3, o1blk, o2, c]
        arr = results[k]["y"].astype(np.float32).reshape(
            2, 2, 2, 16, 64, 2, 64, C
        )
        # (s1 0, s3 1, s2 2, o1s 3, o3 4, o1blk 5, o2 6, c 7)
        #   -> (o1blk, o1s, o2, o3, s1, s2, s3, c)
        out[b, 32 * h:32 * h + 32] = (
            arr.transpose(5, 3, 6, 4, 0, 2, 1, 7).reshape(32, 64, 64, 8 * C)
        )
    return out


def kernel(**inputs):
    x = np.ascontiguousarray(np.asarray(inputs["inputs"], dtype=np.float32))
    A = np.asarray(inputs["A"], dtype=np.float32)
    assert x.shape == (B, N, N, N, C), x.shape

    if not np.allclose(A, _haar_matrix(), atol=1e-5):
        # Kernel hardcodes the 2-tap Haar structure; fall back for generic A.
        return _reference_numpy(x, A)

    from concourse.bass_utils import run_bass_kernel_spmd

    nc, in_maps = _prepare(x, A)
    res = run_bass_kernel_spmd(nc, in_maps, core_ids=list(range(N_CORES)))
    return _assemble(res.results)


# revision 40
# speedup vs baseline: 1.1828x; 1.1757x over previous
"""3D Haar DWT (clean-mode subband stack) on 8 Trainium2 NeuronCores.

Problem (hardcoded): inputs (4, 128, 128, 128, 4) f32, A (128, 128) f32 Haar
analysis operator. Output (4, 64, 64, 64, 32) f32 = 8 subbands stacked on the
channel axis (LLL, LLH, LHL, LHH, HLL, HLH, HHL, HHH) x 4 channels.

Sharding: pure data parallel over (batch, d1-half): core k handles
b = k // 2, d1 range [64*(k%2), 64*(k%2)+64). The Haar transform is a 2-tap
non-overlapping filter, so splitting d1 on an even boundary requires no
communication.

The kernel is memory-bound (HBM ~358 GB/s per core), so the datapath runs in
bf16: the host uploads the input slab as bf16 (8 MiB/core instead of 16) and
the output is stored as bf16 (8 MiB instead of 16), halving HBM traffic vs
f32. absmax-relative error lands ~4e-3, inside the 2e-2 gate.

Key structure: the partition axis carries (o1_sub 16, d1-member, d3-parity,
d2-member) = 128, so a SINGLE PE pass applies ALL THREE Haar butterflies at
once — the stationary matrix is the triple Kronecker of the 2-tap stages
(entries +-0.25, bf16-exact, 8 nonzeros per output, loaded once). The PSUM
partition axis comes out subband-major: (s1, s3, s2, o1_sub). The ONLY
elementwise work left is the mandatory one-input PSUM evacuation (the ISA
allows at most one PSUM operand per elementwise op and DMA cannot read
PSUM), split 50/50 between ACT and DVE (~19 us each) — every engine then
sits far below the ~42 us DMA stream, so the pipeline is DMA-ring-paced
end to end.

Per-core pipeline (host layout [(o1s,m1,m3,m2) = 128, o3 64, o1blk 2,
o2 64, c 4]):
  1. DMA in 2 MiB loads (16 KiB descriptor runs — measured 399 GB/s vs 326
     at 1 MiB) on the SP HWDGE ring; all loads enqueued before any store so
     stores can never head-of-line-block a load.
  2. PE: 8 x 512-col matmuls per 8-o3 block with the Kronecker stationary
     (one PSUM bank each; the f32-PSUM moving-operand ISA cap is 512 cols).
  3. PSUM evacuation (one-input copy, f32 -> bf16) into the store staging
     tile, one op per half-block, alternating ACT / DVE.
  4. One 1 MiB store per block (8 KiB runs) on the SP ring; the output
     partition axis is already subband-major, so the host just casts and
     transposes.

Scale bookkeeping: reference applies s = 1/sqrt(2) per axis (s^3 total). The
host pre-scales by sqrt(2) and the PE applies 0.25: sqrt(2)/4 = s^3 — exact.
All three butterflies accumulate in f32 PSUM, so the only roundings are the
input cast and the evacuation cast.
"""

import sys

import numpy as np

if "/opt/trn_rl_repo" not in sys.path:
    sys.path.insert(0, "/opt/trn_rl_repo")

B, N, C = 4, 128, 4
N_CORES = 8
SLAB = 64          # d1 extent per core
LO3 = 16           # o3 values per load (2 MiB transfers, 16 KiB runs)
O3C = 8            # o3 values per compute block / store (1 MiB stores)
NLOAD = 64 // LO3
NBLK = 64 // O3C

_BASS_CACHE = {}


def _haar_matrix():
    s = np.float32(1.0 / np.sqrt(2.0))
    A = np.zeros((N, N), dtype=np.float32)
    for i in range(N // 2):
        A[i, 2 * i] = s
        A[i, 2 * i + 1] = s
        A[64 + i, 2 * i] = -s
        A[64 + i, 2 * i + 1] = s
    return A


def _kron_weights():
    """lhsT [p_in, p_out] for the combined (d1, d3, d2) butterfly.

    p_in  = 8 * o1s + 4 * m1 + 2 * m3 + m2   (input partition order)
    p_out = 64 * s1 + 32 * s3 + 16 * s2 + o1s (output partition order)
    weight = 0.25 * g(s1, m1) * g(s3, m3) * g(s2, m2),
    g(0, m) = +1, g(1, 0) = -1, g(1, 1) = +1 (Haar lo = a+b, hi = b-a).
    """
    g = np.array([[1.0, 1.0], [-1.0, 1.0]], dtype=np.float32)
    lhsT = np.zeros((N, N), dtype=np.float32)
    for o1s in range(16):
        for m1 in range(2):
            for m3 in range(2):
                for m2 in range(2):
                    p_in = 8 * o1s + 4 * m1 + 2 * m3 + m2
                    for s1 in range(2):
                        for s3 in range(2):
                            for s2 in range(2):
                                p_out = 64 * s1 + 32 * s3 + 16 * s2 + o1s
                                lhsT[p_in, p_out] = (
                                    0.25 * g[s1, m1] * g[s3, m3] * g[s2, m2]
                                )
    return lhsT


def _reference_numpy(inputs, A):
    # Fallback only: exact reference math on host (used if A is not Haar).
    x = np.einsum("ij,bpjqc->bpiqc", A, inputs)
    x = np.einsum("ij,bjpqc->bipqc", A, x)
    x = np.einsum("ij,bpqjc->bpqic", A, x)
    m = x.shape[1] // 2
    subs = [
        x[:, :m, :m, :m, :], x[:, :m, :m, m:, :],
        x[:, :m, m:, :m, :], x[:, :m, m:, m:, :],
        x[:, m:, :m, :m, :], x[:, m:, :m, m:, :],
        x[:, m:, m:, :m, :], x[:, m:, m:, m:, :],
    ]
    return np.concatenate(subs, axis=-1).astype(np.float32)


def _build_bass():
    import concourse.bacc as bacc
    import concourse.mybir as mybir
    import concourse.tile as tile

    f32 = mybir.dt.float32
    bf16 = mybir.dt.bfloat16

    # Bacc (not raw Bass): its compile() pipeline splits multi-sem waits into
    # EventSemaphore instructions — TRN2 instructions have one wait slot.
    nc = bacc.Bacc("TRN2", target_bir_lowering=False, debug=False)
    # x host layout: [(o1s,m1,m3,m2), o3, o1blk, o2, c]; each load descriptor
    # covers a 16 KiB contiguous run per partition.
    x = nc.dram_tensor("x", [N, 64, 2, 64, C], bf16, kind="ExternalInput")
    wk = nc.dram_tensor("wk", [N, N], bf16, kind="ExternalInput")
    # y: [(s1, s3, s2, o1s) = 128, o3, o1blk, o2, c]; per-partition
    # contiguous run for one block's o3 range = 8 KiB.
    y = nc.dram_tensor("y", [N, 64, 2, 64, C], bf16, kind="ExternalOutput")

    with tile.TileContext(nc) as tc:
        with (
            tc.tile_pool(name="const", bufs=1) as cpool,
            tc.tile_pool(name="io", bufs=4) as tpool,
            tc.tile_pool(name="wmid", bufs=4) as wpool,
            tc.tile_pool(name="psum", bufs=4, space="PSUM") as ppool,
        ):
            wk_sb = cpool.tile([N, N], bf16)

            # 1. all loads enqueued up-front on the SP ring: none depends on
            # compute (pool depth covers every load), so the load stream
            # runs back-to-back from the end of the preamble. The first load
            # is 1 MiB so the PE starts ~2.5 us earlier — in fast-DMA device
            # states the drain is PE-paced, so the whole compute stream
            # shifting earlier closes the trailing DMA gaps.
            spans = [(0, 8), (8, 16), (16, 32), (32, 48), (48, 64)]
            Ts = []
            for li, (a, b) in enumerate(spans):
                T = tpool.tile([N, b - a, 2 * 64 * C], bf16, tag=f"T{b - a}")
                nc.sync.dma_start(
                    out=T[:],
                    in_=x[:, a:b].rearrange("p a k q c -> p a (k q c)"),
                )
                Ts.append((a, b, T))
                if li == 0:
                    nc.sync.dma_start(out=wk_sb[:], in_=wk[:, :])

            evac_t = 0
            for ci in range(NBLK):
                a, b, T = next(s for s in Ts if s[0] <= ci * O3C < s[1])
                off = ci * O3C - a
                # W: store staging, (p_out, o3, o1blk*o2*c) bf16.
                W = wpool.tile([N, O3C, 2 * 64 * C], bf16, tag="W")
                for hq in range(4):
                    # 2. all three butterflies as one matmul per o3 value:
                    # 512 cols -> one PSUM bank. PSUM tiles are 2 banks x
                    # 4 bufs: with 4 tiles in flight the evacuations
                    # (alternating engines) always keep up with the PE, so
                    # the matmul cadence never degrades to the evac rate.
                    ps = ppool.tile([N, 2, 512], f32, tag="ps")
                    for j in range(2):
                        nc.tensor.matmul(
                            ps[:, j],
                            lhsT=wk_sb[:],
                            rhs=T[:, off + 2 * hq + j],
                            start=True, stop=True,
                        )
                    # 3. one-input PSUM evacuation (f32 -> bf16) straight
                    # into the store staging tile, alternating ACT / DVE
                    # (both ~1.1 ns/elem per partition; 50/50 keeps either
                    # engine at ~19 us, far below the DMA stream).
                    dst = W[:, 2 * hq:2 * hq + 2]
                    if evac_t % 2 == 1:
                        nc.vector.tensor_copy(out=dst, in_=ps[:])
                    else:
                        nc.scalar.copy(out=dst, in_=ps[:])
                    evac_t += 1

                # 4. one 1 MiB store per block (8 KiB runs/partition) on the
                # SP ring, enqueued behind all loads. Serialized read-phase
                # then write-phase on one FIFO ring measured faster than
                # overlapping R/W on two rings (53.0 vs 57.2 us at equal
                # device rates — mixed traffic gains nothing and the
                # ACT-ring store dispatch trails compute).
                nc.sync.dma_start(
                    out=y[:, ci * O3C:(ci + 1) * O3C].rearrange(
                        "p a k q c -> p a (k q c)"
                    ),
                    in_=W[:],
                )
    nc.compile()
    return nc


def _prepare(x, A):
    """Host-side prep shared with test.py: build (nc, in_maps)."""
    import ml_dtypes

    if "nc" not in _BASS_CACHE:
        _BASS_CACHE["nc"] = _build_bass()
    nc = _BASS_CACHE["nc"]

    wk = np.ascontiguousarray(_kron_weights().astype(ml_dtypes.bfloat16))
    # pre-scale by sqrt(2): the PE applies 0.25 across the three butterflies,
    # so each path nets sqrt(2)/4 = (1/sqrt(2))^3.
    xb = (x * np.float32(np.sqrt(2.0))).astype(ml_dtypes.bfloat16)
    in_maps = []
    for k in range(N_CORES):
        b, h = divmod(k, 2)
        # slab [d1l 64, d2 128, d3 128, c] ->
        # [(o1s, m1, m3, m2) 128, o3 64, o1blk 2, o2 64, c]
        s = xb[b, h * SLAB:(h + 1) * SLAB]
        s = s.reshape(2, 16, 2, 64, 2, 64, 2, C)
        # axes: (o1blk 0, o1s 1, m1 2, o2 3, m2 4, o3 5, m3 6, c 7)
        s = s.transpose(1, 2, 6, 4, 5, 0, 3, 7)
        in_maps.append(
            {
                "x": np.ascontiguousarray(s.reshape(N, 64, 2, 64, C)),
                "wk": wk,
            }
        )
    return nc, in_maps


def _assemble(results):
    """Gather per-core bf16 y tensors into the full f32 output."""
    out = np.empty((B, 64, 64, 64, 8 * C), np.float32)
    for k in range(N_CORES):
        b, h = divmod(k, 2)
        # y: [(s1, s3, s2, o1s), o3, o1blk, o2, c]
        arr = results[k]["y"].astype(np.float32).reshape(
            2, 2, 2, 16, 64, 2, 64, C
        )
        # (s1 0, s3 1, s2 2, o1s 3, o3 4, o1blk 5, o2 6, c 7)
        #   -> (o1blk, o1s, o2, o3, s1, s2, s3, c)
        out[b, 32 * h:32 * h + 32] = (
            arr.transpose(5, 3, 6, 4, 0, 2, 1, 7).reshape(32, 64, 64, 8 * C)
        )
    return out


def kernel(**inputs):
    x = np.ascontiguousarray(np.asarray(inputs["inputs"], dtype=np.float32))
    A = np.asarray(inputs["A"], dtype=np.float32)
    assert x.shape == (B, N, N, N, C), x.shape

    if not np.allclose(A, _haar_matrix(), atol=1e-5):
        # Kernel hardcodes the 2-tap Haar structure; fall back for generic A.
        return _reference_numpy(x, A)

    from concourse.bass_utils import run_bass_kernel_spmd

    nc, in_maps = _prepare(x, A)
    res = run_bass_kernel_spmd(nc, in_maps, core_ids=list(range(N_CORES)))
    return _assemble(res.results)
